# revision 18
# baseline (speedup 1.0000x reference)
"""Trainium2 Bass kernel for the NeuroplasticityModule problem.

Math (per reference):
    bdnf = sigmoid(x @ W.T + b)
    xs   = x * pm
    fi   = xs * (xs @ mask.T) * (0.5 / H)
    out  = xs + bdnf * fi
         = xs * (1 + sigmoid(z1) * z2)
where
    z1 = x @ W.T + b
    z2 = x @ (mask * pm[None, :] * 0.5/H).T     (pm and the 0.5/H constant
                                                 folded into the mask weights)

Strategy: data-parallel over batch on 8 NeuronCores (2048 rows/core).
Weight prep happens on host (layout transforms only): both weight matrices
are transposed and fused into one [H, 2*H] "wcat" so the two matmuls per
output tile share a single stationary operand (x.T tiles).  Matmuls run in
fp8e4m3 with DoubleRow perf mode (fp32 accumulate, per-matrix power-of-two
scales); every elementwise op that touches the dominant xs term stays
fp32, so the overall relative error is ~1e-4 (bf16 fallback: ~5e-6).
"""

import os
import sys

for _p in ("/opt/trn_rl_repo", "/root/.axon_site/_ro/trn_rl_repo"):
    if os.path.isdir(_p) and _p not in sys.path:
        sys.path.append(_p)

from contextlib import ExitStack

import ml_dtypes
import numpy as np

import concourse.bass as bass
import concourse.tile as tile
from concourse import bacc, mybir
from concourse.bass_utils import run_bass_kernel_spmd

B, H = 16384, 2048
NCORES = 8
BL = B // NCORES          # 2048 batch rows per core
P = 128                   # partition dim
KT = H // P               # 16 contraction tiles
KA = KT // 2              # 8 DoubleRow contraction steps
MT = BL // P              # 16 batch tiles per core
NJ = H // 512             # 4 output column blocks of 512
DMN_FACTOR = 0.5
SW = 16.0                 # fp8 scale on the W half of wcat
SM = 4096.0               # fp8 scale on the mask half of wcat

F32 = mybir.dt.float32
BF16 = mybir.dt.bfloat16
FP8 = mybir.dt.float8e4

USE_FP8 = os.environ.get("NEURO_KERNEL_BF16", "") != "1"


def _build(with_bias: bool, fp8: bool):
    mdt = FP8 if fp8 else BF16
    DR = mybir.MatmulPerfMode.DoubleRow if fp8 else None

    nc = bacc.Bacc("TRN2", target_bir_lowering=False, num_devices=NCORES)

    x = nc.dram_tensor("x", [BL, H], F32, kind="ExternalInput")
    # xt[m, p, k, f] = x[128*m + f, 128*k + p]  (pre-transposed tiles,
    # partition-major so each per-m slice loads as one full-BW DMA)
    xt = nc.dram_tensor("xt", [MT, P, KT, P], mdt, kind="ExternalInput")
    if fp8:
        # wsw[a, j, p, s, 0:512]   = W.T[256a+128s+p, 512j:512(j+1)] * SW
        # wsw[a, j, p, s, 512:1024] = mask.T*pm*SM*0.5/H [same rows/cols]
        wsw = nc.dram_tensor("wsw", [KA, NJ, P, 2, 1024], mdt,
                             kind="ExternalInput")
    else:
        # wcat[h, 0:H] = W.T ; wcat[h, H + i] = mask[i, h] * pm[h] * 0.5/H
        wcat = nc.dram_tensor("wcat", [H, 2 * H], mdt, kind="ExternalInput")
    pmb = nc.dram_tensor("pmb", [P, H], F32, kind="ExternalInput")
    if with_bias:
        bb = nc.dram_tensor("bb", [P, H], F32, kind="ExternalInput")
    out = nc.dram_tensor("out", [BL, H], F32, kind="ExternalOutput")

    inv_sw = 1.0 / SW if fp8 else 1.0
    inv_sm = 1.0 / SM if fp8 else 1.0

    with tile.TileContext(nc) as tc, ExitStack() as ctx:
        cpool = ctx.enter_context(tc.tile_pool(name="const", bufs=1))
        wpool = ctx.enter_context(tc.tile_pool(name="wcat", bufs=1))
        xpool = ctx.enter_context(tc.tile_pool(name="x", bufs=2))
        xtpool = ctx.enter_context(tc.tile_pool(name="xt", bufs=2))
        opool = ctx.enter_context(tc.tile_pool(name="o", bufs=2))
        spool = ctx.enter_context(tc.tile_pool(name="s", bufs=3))
        pspool = ctx.enter_context(tc.tile_pool(name="ps", bufs=4, space="PSUM"))

        wt = []
        if fp8:
            wt = [[None] * NJ for _ in range(KA)]
            for j in range(NJ):
                for a in range(KA):
                    w = wpool.tile([P, 2, 1024], mdt, tag=f"w{a}_{j}")
                    # j=0 tiles gate the very first matmuls: issue them on
                    # the HW-DGE queue, the bulk on SW-DGE
                    eng = nc.sync if j == 0 else nc.gpsimd
                    eng.dma_start(w[:], wsw[a, j])
                    wt[a][j] = w
        else:
            for k in range(KT):
                w = wpool.tile([P, 2 * H], mdt, tag=f"w{k}")
                nc.gpsimd.dma_start(w[:], wcat[k * P:(k + 1) * P, :])
                wt.append(w)

        # constants load after the weight tiles so they don't delay the
        # critical-path first matmuls (they're first needed ~25us in)
        pmt = cpool.tile([P, H], F32, tag="pmb")
        nc.gpsimd.dma_start(pmt[:], pmb[:])
        if with_bias:
            bbt = cpool.tile([P, H], F32, tag="bb")
            nc.gpsimd.dma_start(bbt[:], bb[:])

        for m in range(MT):
            if fp8:
                xtm = xtpool.tile([P, KT, P], mdt)
                nc.sync.dma_start(xtm[:], xt[m])
            else:
                xtm = xtpool.tile([P, KT * P], mdt)
                nc.sync.dma_start(xtm[:], xt[m].rearrange("p k f -> p (k f)"))
            xm = xpool.tile([P, H], F32)
            nc.gpsimd.dma_start(xm[:], x[m * P:(m + 1) * P, :])

            om = opool.tile([P, H], F32)
            for j in range(NJ):
                js = slice(j * 512, (j + 1) * 512)
                js2 = slice(H + j * 512, H + (j + 1) * 512)
                ps1 = pspool.tile([P, 512], F32, tag="ps1")
                ps2 = pspool.tile([P, 512], F32, tag="ps2")
                if fp8:
                    for a in range(KA):
                        lhsT = xtm[:, 2 * a:2 * a + 2, :]
                        nc.tensor.matmul(ps1[:], lhsT, wt[a][j][:, :, 0:512],
                                         perf_mode=DR,
                                         start=(a == 0), stop=(a == KA - 1))
                        nc.tensor.matmul(ps2[:], lhsT, wt[a][j][:, :, 512:1024],
                                         perf_mode=DR,
                                         start=(a == 0), stop=(a == KA - 1))
                else:
                    for k in range(KT):
                        lhsT = xtm[:, k * P:(k + 1) * P]
                        nc.tensor.matmul(ps1[:], lhsT, wt[k][:, js],
                                         start=(k == 0), stop=(k == KT - 1))
                        nc.tensor.matmul(ps2[:], lhsT, wt[k][:, js2],
                                         start=(k == 0), stop=(k == KT - 1))

                sig = spool.tile([P, 512], F32, tag="sig")
                if with_bias:
                    zb = spool.tile([P, 512], F32, tag="zb")
                    if fp8:
                        # zb = ps1/SW + b
                        nc.scalar.mul(zb[:], ps1[:], inv_sw)
                        nc.vector.tensor_add(zb[:], zb[:], bbt[:, js])
                    else:
                        nc.vector.tensor_add(zb[:], ps1[:], bbt[:, js])
                    nc.scalar.activation(sig[:], zb[:],
                                         mybir.ActivationFunctionType.Sigmoid)
                else:
                    nc.scalar.activation(sig[:], ps1[:],
                                         mybir.ActivationFunctionType.Sigmoid,
                                         scale=inv_sw)
                t = spool.tile([P, 512], F32, tag="t")
                nc.vector.tensor_mul(t[:], sig[:], ps2[:])
                t1 = spool.tile([P, 512], F32, tag="t1")
                # t1 = t * (1/SM) + 1  (one dual-op DVE instruction)
                nc.vector.tensor_scalar(t1[:], t[:], inv_sm, 1.0,
                                        mybir.AluOpType.mult,
                                        mybir.AluOpType.add)
                t2 = spool.tile([P, 512], F32, tag="t2")
                nc.vector.tensor_mul(t2[:], t1[:], pmt[:, js])
                nc.vector.tensor_mul(om[:, js], xm[:, js], t2[:])
                nc.sync.dma_start(out[m * P:(m + 1) * P, js], om[:, js])

    nc.compile()
    return nc


_cache = {}


def _get_nc(with_bias: bool, fp8: bool):
    key = (with_bias, fp8)
    if key not in _cache:
        _cache[key] = _build(with_bias, fp8)
    return _cache[key]


def _prep_inputs(x, W, b, plasticity_modulation, connectivity_mask, fp8):
    pm = np.asarray(plasticity_modulation, dtype=np.float32)
    W = np.asarray(W, dtype=np.float32)
    mask = np.asarray(connectivity_mask, dtype=np.float32)
    x = np.asarray(x, dtype=np.float32)
    b = np.asarray(b, dtype=np.float32)

    with_bias = bool(np.any(b != 0.0))
    ndt = mybir.dt.np(FP8) if fp8 else ml_dtypes.bfloat16
    sw = SW if fp8 else 1.0
    sm = SM if fp8 else 1.0

    # weight prep (host): transpose + fuse pm, 0.5/H and fp8 scales
    Wt = W.T * sw
    Mt = mask.T * (pm * (sm * DMN_FACTOR / H))[:, None]
    if fp8:
        # [KA, NJ, P, 2, 1024]: per-(a, j) contiguous DoubleRow pair tiles
        wcat = np.concatenate([
            Wt.reshape(KA, 2, P, NJ, 512).transpose(0, 3, 2, 1, 4),
            Mt.reshape(KA, 2, P, NJ, 512).transpose(0, 3, 2, 1, 4),
        ], axis=4).astype(ndt)
        wname = "wsw"
    else:
        wcat = np.empty((H, 2 * H), dtype=ndt)
        wcat[:, :H] = Wt.astype(ndt)
        wcat[:, H:] = Mt.astype(ndt)
        wname = "wcat"

    pmb = np.ascontiguousarray(np.broadcast_to(pm[None, :], (P, H)))
    if with_bias:
        bb = np.ascontiguousarray(np.broadcast_to(b[None, :], (P, H)))

    in_maps = []
    for c in range(NCORES):
        xc = x[c * BL:(c + 1) * BL]
        # xt[m, p, k, f] = xc[128m + f, 128k + p]
        xtc = np.ascontiguousarray(
            xc.reshape(MT, P, KT, P).transpose(0, 3, 2, 1)
        ).astype(ndt)
        m = {"x": xc, "xt": xtc, wname: wcat, "pmb": pmb}
        if with_bias:
            m["bb"] = bb
        in_maps.append(m)
    return with_bias, in_maps


def _run(inputs, trace):
    with_bias, in_maps = _prep_inputs(fp8=USE_FP8, **inputs)
    nc = _get_nc(with_bias, USE_FP8)
    res = run_bass_kernel_spmd(nc, in_maps, core_ids=list(range(NCORES)),
                               trace=trace)
    out = np.concatenate([res.results[c]["out"] for c in range(NCORES)], axis=0)
    return out, res


def kernel(x, W, b, plasticity_modulation, connectivity_mask):
    inputs = dict(x=x, W=W, b=b, plasticity_modulation=plasticity_modulation,
                  connectivity_mask=connectivity_mask)
    out, _ = _run(inputs, trace=False)
    if np.isnan(out).any():  # one retry on transient corruption
        out, _ = _run(inputs, trace=False)
    return out


def kernel_traced(x, W, b, plasticity_modulation, connectivity_mask):
    """Like kernel() but also returns the profiled BassKernelResults."""
    inputs = dict(x=x, W=W, b=b, plasticity_modulation=plasticity_modulation,
                  connectivity_mask=connectivity_mask)
    return _run(inputs, trace=True)


# revision 19
# speedup vs baseline: 1.0393x; 1.0393x over previous
"""Trainium2 Bass kernel for the NeuroplasticityModule problem.

Math (per reference):
    bdnf = sigmoid(x @ W.T + b)
    xs   = x * pm
    fi   = xs * (xs @ mask.T) * (0.5 / H)
    out  = xs + bdnf * fi
         = xs * (1 + sigmoid(z1) * z2)
where
    z1 = x @ W.T + b
    z2 = x @ (mask * pm[None, :] * 0.5/H).T     (pm and the 0.5/H constant
                                                 folded into the mask weights)

Strategy: data-parallel over batch on 8 NeuronCores (2048 rows/core).
Weight prep happens on host (layout transforms only): both weight matrices
are transposed and fused into one [H, 2*H] "wcat" so the two matmuls per
output tile share a single stationary operand (x.T tiles).  Matmuls run in
fp8e4m3 with DoubleRow perf mode (fp32 accumulate, per-matrix power-of-two
scales); every elementwise op that touches the dominant xs term stays
fp32, so the overall relative error is ~1e-4 (bf16 fallback: ~5e-6).
"""

import os
import sys

for _p in ("/opt/trn_rl_repo", "/root/.axon_site/_ro/trn_rl_repo"):
    if os.path.isdir(_p) and _p not in sys.path:
        sys.path.append(_p)

from contextlib import ExitStack

import ml_dtypes
import numpy as np

import concourse.bass as bass
import concourse.tile as tile
from concourse import bacc, mybir
from concourse.bass_utils import run_bass_kernel_spmd

B, H = 16384, 2048
NCORES = 8
BL = B // NCORES          # 2048 batch rows per core
P = 128                   # partition dim
KT = H // P               # 16 contraction tiles
KA = KT // 2              # 8 DoubleRow contraction steps
MT = BL // P              # 16 batch tiles per core
NJ = H // 512             # 4 output column blocks of 512
DMN_FACTOR = 0.5
SW = 16.0                 # fp8 scale on the W half of wcat
SM = 4096.0               # fp8 scale on the mask half of wcat

F32 = mybir.dt.float32
BF16 = mybir.dt.bfloat16
FP8 = mybir.dt.float8e4

USE_FP8 = os.environ.get("NEURO_KERNEL_BF16", "") != "1"


def _build(with_bias: bool, fp8: bool):
    mdt = FP8 if fp8 else BF16
    DR = mybir.MatmulPerfMode.DoubleRow if fp8 else None

    nc = bacc.Bacc("TRN2", target_bir_lowering=False, num_devices=NCORES)

    x = nc.dram_tensor("x", [BL, H], F32, kind="ExternalInput")
    # xt[m, p, k, f] = x[128*m + f, 128*k + p]  (pre-transposed tiles,
    # partition-major so each per-m slice loads as one full-BW DMA)
    xt = nc.dram_tensor("xt", [MT, P, KT, P], mdt, kind="ExternalInput")
    if fp8:
        # wsw[a, j, p, s, 0:512]   = W.T[256a+128s+p, 512j:512(j+1)] * SW
        # wsw[a, j, p, s, 512:1024] = mask.T*pm*SM*0.5/H [same rows/cols]
        wsw = nc.dram_tensor("wsw", [KA, NJ, P, 2, 1024], mdt,
                             kind="ExternalInput")
    else:
        # wcat[h, 0:H] = W.T ; wcat[h, H + i] = mask[i, h] * pm[h] * 0.5/H
        wcat = nc.dram_tensor("wcat", [H, 2 * H], mdt, kind="ExternalInput")
    pmb = nc.dram_tensor("pmb", [P, H], F32, kind="ExternalInput")
    if with_bias:
        bb = nc.dram_tensor("bb", [P, H], F32, kind="ExternalInput")
    out = nc.dram_tensor("out", [BL, H], F32, kind="ExternalOutput")

    inv_sw = 1.0 / SW if fp8 else 1.0
    inv_sm = 1.0 / SM if fp8 else 1.0

    with tile.TileContext(nc) as tc, ExitStack() as ctx:
        cpool = ctx.enter_context(tc.tile_pool(name="const", bufs=1))
        wpool = ctx.enter_context(tc.tile_pool(name="wcat", bufs=1))
        xpool = ctx.enter_context(tc.tile_pool(name="x", bufs=2))
        xtpool = ctx.enter_context(tc.tile_pool(name="xt", bufs=2))
        opool = ctx.enter_context(tc.tile_pool(name="o", bufs=2))
        spool = ctx.enter_context(tc.tile_pool(name="s", bufs=3))
        pspool = ctx.enter_context(tc.tile_pool(name="ps", bufs=4, space="PSUM"))

        wt = []
        if fp8:
            wt = [[None] * NJ for _ in range(KA)]
            for j in range(NJ):
                for a in range(KA):
                    w = wpool.tile([P, 2, 1024], mdt, tag=f"w{a}_{j}")
                    nc.gpsimd.dma_start(w[:], wsw[a, j])
                    wt[a][j] = w
        else:
            for k in range(KT):
                w = wpool.tile([P, 2 * H], mdt, tag=f"w{k}")
                nc.gpsimd.dma_start(w[:], wcat[k * P:(k + 1) * P, :])
                wt.append(w)

        # constants load after the weight tiles so they don't delay the
        # critical-path first matmuls (they're first needed ~25us in)
        pmt = cpool.tile([P, H], F32, tag="pmb")
        nc.gpsimd.dma_start(pmt[:], pmb[:])
        if with_bias:
            bbt = cpool.tile([P, H], F32, tag="bb")
            nc.gpsimd.dma_start(bbt[:], bb[:])

        for m in range(MT):
            if fp8:
                xtm = xtpool.tile([P, KT, P], mdt)
                nc.sync.dma_start(xtm[:], xt[m])
            else:
                xtm = xtpool.tile([P, KT * P], mdt)
                nc.sync.dma_start(xtm[:], xt[m].rearrange("p k f -> p (k f)"))
            xm = xpool.tile([P, H], F32)
            nc.gpsimd.dma_start(xm[:], x[m * P:(m + 1) * P, :])

            om = opool.tile([P, H], F32)
            for j in range(NJ):
                js = slice(j * 512, (j + 1) * 512)
                js2 = slice(H + j * 512, H + (j + 1) * 512)
                ps1 = pspool.tile([P, 512], F32, tag="ps1")
                ps2 = pspool.tile([P, 512], F32, tag="ps2")
                if fp8:
                    for a in range(KA):
                        lhsT = xtm[:, 2 * a:2 * a + 2, :]
                        nc.tensor.matmul(ps1[:], lhsT, wt[a][j][:, :, 0:512],
                                         perf_mode=DR,
                                         start=(a == 0), stop=(a == KA - 1))
                        nc.tensor.matmul(ps2[:], lhsT, wt[a][j][:, :, 512:1024],
                                         perf_mode=DR,
                                         start=(a == 0), stop=(a == KA - 1))
                else:
                    for k in range(KT):
                        lhsT = xtm[:, k * P:(k + 1) * P]
                        nc.tensor.matmul(ps1[:], lhsT, wt[k][:, js],
                                         start=(k == 0), stop=(k == KT - 1))
                        nc.tensor.matmul(ps2[:], lhsT, wt[k][:, js2],
                                         start=(k == 0), stop=(k == KT - 1))

                sig = spool.tile([P, 512], F32, tag="sig")
                if with_bias:
                    zb = spool.tile([P, 512], F32, tag="zb")
                    if fp8:
                        # zb = ps1/SW + b
                        nc.scalar.mul(zb[:], ps1[:], inv_sw)
                        nc.vector.tensor_add(zb[:], zb[:], bbt[:, js])
                    else:
                        nc.vector.tensor_add(zb[:], ps1[:], bbt[:, js])
                    nc.scalar.activation(sig[:], zb[:],
                                         mybir.ActivationFunctionType.Sigmoid)
                else:
                    nc.scalar.activation(sig[:], ps1[:],
                                         mybir.ActivationFunctionType.Sigmoid,
                                         scale=inv_sw)
                t = spool.tile([P, 512], F32, tag="t")
                nc.vector.tensor_mul(t[:], sig[:], ps2[:])
                t1 = spool.tile([P, 512], F32, tag="t1")
                # t1 = t * (1/SM) + 1  (one dual-op DVE instruction)
                nc.vector.tensor_scalar(t1[:], t[:], inv_sm, 1.0,
                                        mybir.AluOpType.mult,
                                        mybir.AluOpType.add)
                t2 = spool.tile([P, 512], F32, tag="t2")
                nc.vector.tensor_mul(t2[:], t1[:], pmt[:, js])
                nc.vector.tensor_mul(om[:, js], xm[:, js], t2[:])
                nc.sync.dma_start(out[m * P:(m + 1) * P, js], om[:, js])

    nc.compile()
    return nc


_cache = {}


def _get_nc(with_bias: bool, fp8: bool):
    key = (with_bias, fp8)
    if key not in _cache:
        _cache[key] = _build(with_bias, fp8)
    return _cache[key]


def _prep_inputs(x, W, b, plasticity_modulation, connectivity_mask, fp8):
    pm = np.asarray(plasticity_modulation, dtype=np.float32)
    W = np.asarray(W, dtype=np.float32)
    mask = np.asarray(connectivity_mask, dtype=np.float32)
    x = np.asarray(x, dtype=np.float32)
    b = np.asarray(b, dtype=np.float32)

    with_bias = bool(np.any(b != 0.0))
    ndt = mybir.dt.np(FP8) if fp8 else ml_dtypes.bfloat16
    sw = SW if fp8 else 1.0
    sm = SM if fp8 else 1.0

    # weight prep (host): transpose + fuse pm, 0.5/H and fp8 scales
    Wt = W.T * sw
    Mt = mask.T * (pm * (sm * DMN_FACTOR / H))[:, None]
    if fp8:
        # [KA, NJ, P, 2, 1024]: per-(a, j) contiguous DoubleRow pair tiles
        wcat = np.concatenate([
            Wt.reshape(KA, 2, P, NJ, 512).transpose(0, 3, 2, 1, 4),
            Mt.reshape(KA, 2, P, NJ, 512).transpose(0, 3, 2, 1, 4),
        ], axis=4).astype(ndt)
        wname = "wsw"
    else:
        wcat = np.empty((H, 2 * H), dtype=ndt)
        wcat[:, :H] = Wt.astype(ndt)
        wcat[:, H:] = Mt.astype(ndt)
        wname = "wcat"

    pmb = np.ascontiguousarray(np.broadcast_to(pm[None, :], (P, H)))
    if with_bias:
        bb = np.ascontiguousarray(np.broadcast_to(b[None, :], (P, H)))

    in_maps = []
    for c in range(NCORES):
        xc = x[c * BL:(c + 1) * BL]
        # xt[m, p, k, f] = xc[128m + f, 128k + p]
        xtc = np.ascontiguousarray(
            xc.reshape(MT, P, KT, P).transpose(0, 3, 2, 1)
        ).astype(ndt)
        m = {"x": xc, "xt": xtc, wname: wcat, "pmb": pmb}
        if with_bias:
            m["bb"] = bb
        in_maps.append(m)
    return with_bias, in_maps


def _run(inputs, trace):
    with_bias, in_maps = _prep_inputs(fp8=USE_FP8, **inputs)
    nc = _get_nc(with_bias, USE_FP8)
    res = run_bass_kernel_spmd(nc, in_maps, core_ids=list(range(NCORES)),
                               trace=trace)
    out = np.concatenate([res.results[c]["out"] for c in range(NCORES)], axis=0)
    return out, res


def kernel(x, W, b, plasticity_modulation, connectivity_mask):
    inputs = dict(x=x, W=W, b=b, plasticity_modulation=plasticity_modulation,
                  connectivity_mask=connectivity_mask)
    out, _ = _run(inputs, trace=False)
    if np.isnan(out).any():  # one retry on transient corruption
        out, _ = _run(inputs, trace=False)
    return out


def kernel_traced(x, W, b, plasticity_modulation, connectivity_mask):
    """Like kernel() but also returns the profiled BassKernelResults."""
    inputs = dict(x=x, W=W, b=b, plasticity_modulation=plasticity_modulation,
                  connectivity_mask=connectivity_mask)
    return _run(inputs, trace=True)
